# revision 34
# baseline (speedup 1.0000x reference)
"""Multi-head self-attention (B=4, T=2048, C=1024, H=16 heads, causal) on 8 TRN2
NeuronCores, head-tensor-parallel.

Per core c (owning heads 2c, 2c+1 = attn feature rows [c*128,(c+1)*128)):
  1. QKV for all 4 batches (bf16 operands; x and w_qkv pre-converted on host):
     qT/kT feature-major bf16; v is PE-transposed to natural [tok, feat] tiles
     twice (bf16 for the diagonal PV, fp8e4m3 for the off-diagonal DoubleRow
     PV), each with ones columns appended per head so PV also emits the
     softmax denominator row.
  2. Causal attention: scoresT [kv, q] via row-tiled (K=64) bf16 matmul pairs
     (both heads in one 2-bank PSUM tile). exp(x*scale - 1) on ScalarE (the -1
     keeps fp8 prob tiles in e4m3 range; it cancels in the normalization).
     Off-diagonal kv tiles: probs as fp8, PV via DoubleRow fp8 matmuls (two kv
     tiles per instruction). Diagonal kv tiles: bf16 probs, columns trimmed to
     the causal region, [128,128] triangle mask multiply, bf16 PV.
  3. Normalization: reciprocal_approx_fast of the denominator row (staged via
     SBUF), DMA-broadcast across partitions, DVE multiply straight out of PSUM.
  4. Two passes (even q-blocks then odd); AllToAll per pass; projection of a
     token half hides under the second AllToAll.
  5. Output projection (bf16) + bias for this core's 1024-token slice.
Host gathers the 8 [1024 feat, 1024 tok] slices, concatenates and transposes.
"""
import ml_dtypes
import numpy as np

import concourse.bass as bass
import concourse.tile as tile
from concourse import bacc, mybir
from concourse.bass_utils import run_bass_kernel_spmd

F32 = mybir.dt.float32
BF16 = mybir.dt.bfloat16
F8 = mybir.dt.float8e4

B, T, C = 4, 2048, 1024
N_HEADS, HEAD = 16, 64
N_CORES = 8
BT = B * T
TOK_PER_CORE = BT // N_CORES    # 1024
TB = 512                        # token block (matmul moving dim)
NKT = C // 128                  # 8 contraction tiles
SCALE = HEAD ** -0.5
EXP_BIAS = -1.0                 # exp(s*scale - 1); cancels in normalization


def build():
    nc = bacc.Bacc("TRN2", target_bir_lowering=False, debug=False, num_devices=N_CORES)

    xT = nc.dram_tensor("xT", [C, BT], BF16, kind="ExternalInput")
    wqkvT = nc.dram_tensor("wqkvT", [C, 384], BF16, kind="ExternalInput")
    wprojT = nc.dram_tensor("wprojT", [C, C], BF16, kind="ExternalInput")
    bmat = nc.dram_tensor("bmat", [128, 8], F32, kind="ExternalInput")
    tri_in = nc.dram_tensor("tri", [128, 128], BF16, kind="ExternalInput")
    ident_in = nc.dram_tensor("ident", [128, 128], BF16, kind="ExternalInput")
    ident8_in = nc.dram_tensor("ident8", [128, 128], F8, kind="ExternalInput")

    outT = nc.dram_tensor("outT", [C, TOK_PER_CORE], F32, kind="ExternalOutput")

    # half X = even q-blocks (local token halves 0), half Y = odd q-blocks
    qT_d = nc.dram_tensor("qT_d", [128, BT], BF16)
    rnorm_d = nc.dram_tensor("rnorm_d", [16, 2 * TB], F32)
    a2i = [nc.dram_tensor(f"a2i{h}", [N_CORES, 128, TB], BF16) for h in range(2)]
    a2o = [nc.dram_tensor(f"a2o{h}", [N_CORES, 128, TB], BF16) for h in range(2)]

    xT_r = xT.ap().rearrange("(kt p) n -> p kt n", p=128)

    with tile.TileContext(nc) as tc:
        with (
            tc.tile_pool(name="consts", bufs=1) as consts,
            tc.tile_pool(name="wp", bufs=1) as wp_pool,
            tc.tile_pool(name="xt", bufs=3) as xt_pool,
            tc.tile_pool(name="qk", bufs=4) as qk_pool,
            tc.tile_pool(name="qst", bufs=3) as qst_pool,
            tc.tile_pool(name="vnat", bufs=4) as vnat_pool,
            tc.tile_pool(name="vte", bufs=2) as vte_pool,
            tc.tile_pool(name="exp", bufs=4) as exp_pool,
            tc.tile_pool(name="evac", bufs=3) as evac_pool,
            tc.tile_pool(name="sr", bufs=3) as sr_pool,
            tc.tile_pool(name="po", bufs=2) as po_pool,
            tc.tile_pool(name="ps_a", bufs=2, space="PSUM") as ps_a,  # qkv/pv/proj
            tc.tile_pool(name="ps_b", bufs=2, space="PSUM") as ps_b,  # scores/transp
        ):
            wqkv_sb = consts.tile([128, NKT, 384], BF16)
            wqkvT_r = wqkvT.ap().rearrange("(kt p) m -> p kt m", p=128)
            for kt in range(NKT):
                nc.sync.dma_start(out=wqkv_sb[:, kt, :], in_=wqkvT_r[:, kt, :])
            ident = consts.tile([128, 128], BF16)
            nc.sync.dma_start(out=ident, in_=ident_in.ap())
            ident8 = consts.tile([128, 128], F8)
            nc.sync.dma_start(out=ident8, in_=ident8_in.ap())
            ebias = consts.tile([128, 1], F32)
            nc.vector.memset(ebias[:], EXP_BIAS)

            kTs, vns, v8s = [], [], []

            # ---- QKV projections, all batches ----
            for b in range(B):
                tok0 = b * T
                kT = qk_pool.tile([128, T], BF16, tag="kT")
                v_nat = vnat_pool.tile([128, 16, 130], BF16, tag="vnat")
                # interleaved dual-fp8 weight layout: [pair, col s, tile]
                # storage col s holds logical out-row (65-s): s=0 pad, s=1 ones
                vA8 = vnat_pool.tile([128, 8, 128, 2], F8, tag="vA8")
                vB8 = vnat_pool.tile([128, 8, 128, 2], F8, tag="vB8")
                nc.vector.memset(v_nat[:, :, 64], 1.0)
                nc.vector.memset(v_nat[:, :, 129], 1.0)
                for v8h in (vA8, vB8):
                    nc.vector.memset(v8h[:, :, 0:63, :], 0.0)
                    nc.vector.memset(v8h[:, :, 63, :], 1.0)
                kTs.append(kT); vns.append(v_nat); v8s.append((vA8, vB8))

                for tb in range(T // TB):
                    col0 = tok0 + tb * TB
                    xt = xt_pool.tile([128, NKT, TB], BF16, tag="xt")
                    for kt in range(NKT):
                        nc.sync.dma_start(out=xt[:, kt, :], in_=xT_r[:, kt, col0:col0 + TB])
                    for m in range(3):  # 0=q, 1=k, 2=v (feature-major)
                        ps = ps_a.tile([128, TB], F32, tag="a")
                        for kt in range(NKT):
                            nc.tensor.matmul(
                                ps[:],
                                lhsT=wqkv_sb[:, kt, m * 128:(m + 1) * 128],
                                rhs=xt[:, kt, :],
                                start=(kt == 0),
                                stop=(kt == NKT - 1),
                            )
                        sl = slice(tb * TB, (tb + 1) * TB)
                        if m == 0:
                            qo = vte_pool.tile([128, TB], BF16, tag="qo")
                            nc.scalar.copy(qo[:], ps[:])
                            nc.sync.dma_start(out=qT_d.ap()[:, col0:col0 + TB], in_=qo[:])
                        elif m == 1:
                            nc.vector.tensor_copy(kT[:, sl], ps[:])
                        else:
                            vte = vte_pool.tile([128, TB], BF16, tag="vte")
                            nc.scalar.copy(vte[:], ps[:])
                            vte8 = vte_pool.tile([128, TB], F8, tag="vte8")
                            with nc.allow_low_precision(reason="off-diag PV in fp8"):
                                nc.scalar.copy(vte8[:], ps[:])
                            for q in range(TB // 128):
                                jt = tb * 4 + q
                                csl = slice(q * 128, (q + 1) * 128)
                                ps_tr = ps_b.tile([128, 128], BF16, tag="s")
                                nc.tensor.transpose(ps_tr[:], vte[:, csl], ident[:])
                                nc.vector.tensor_copy(v_nat[:, jt, 0:64], ps_tr[:, 0:64])
                                nc.vector.tensor_copy(v_nat[:, jt, 65:129], ps_tr[:, 64:128])
                                # fp8 transpose (col-reversed ident, step-2 out)
                                ps_tr8 = ps_b.tile([128, 128, 2], F8, tag="s")
                                nc.tensor.transpose(ps_tr8[:, :, 0], vte8[:, csl], ident8[:])
                                nc.vector.tensor_copy(
                                    vA8[:, jt // 2, 64:128, jt % 2], ps_tr8[:, 64:128, 0])
                                nc.vector.tensor_copy(
                                    vB8[:, jt // 2, 64:128, jt % 2], ps_tr8[:, 0:64, 0])

            tri_sb = consts.tile([128, 128], BF16)
            nc.sync.dma_start(out=tri_sb, in_=tri_in.ap())
            bmat_sb = consts.tile([128, 8], F32)
            nc.sync.dma_start(out=bmat_sb, in_=bmat.ap())
            wproj_sb = wp_pool.tile([128, NKT, C], BF16)
            nc.sync.dma_start(out=wproj_sb, in_=wprojT.ap().rearrange("(kt p) m -> p kt m", p=128))

            # ---- causal attention, two passes over q-blocks ----
            def attn_compute(b, ib, half):
                kT, v_nat = kTs[b], vns[b]
                vA8, vB8 = v8s[b]
                qt = qst_pool.tile([128, TB], BF16, tag="qst")
                qcol = b * T + ib * TB
                nc.sync.dma_start(out=qt, in_=qT_d.ap()[:, qcol:qcol + TB])
                pv = ps_a.tile([128, 2 * TB], F32, tag="a")

                # off-diagonal kv tiles: fp8 probs, DoubleRow PV (2 kv tiles/mm)
                for pr in range(ib * 2):
                    jt0 = pr * 2
                    e2 = exp_pool.tile([128, 2, 2, TB], F8, tag="e2")
                    for j in range(2):
                        jsl = slice((jt0 + j) * 128, (jt0 + j + 1) * 128)
                        s = ps_b.tile([128, 2 * TB], F32, tag="s")
                        nc.tensor.matmul(
                            s[:, 0:TB], lhsT=kT[0:64, jsl], rhs=qt[0:64, :],
                            start=True, stop=True, tile_position=(0, 0),
                        )
                        nc.tensor.matmul(
                            s[:, TB:2 * TB], lhsT=kT[64:128, jsl], rhs=qt[64:128, :],
                            start=True, stop=True, tile_position=(64, 0),
                        )
                        with nc.allow_low_precision(reason="off-diag probs in fp8"):
                            nc.scalar.activation(
                                e2[:, j, :, :], s[:],
                                mybir.ActivationFunctionType.Exp,
                                bias=ebias[:], scale=SCALE,
                            )
                    nc.tensor.matmul(
                        pv[0:128, 0:TB], lhsT=vA8[:, pr, :, :],
                        rhs=e2[:, :, 0, :],
                        start=(pr == 0), stop=False,
                        perf_mode=mybir.MatmulPerfMode.DoubleRowSwInterleave,
                        skip_group_check=True,
                    )
                    nc.tensor.matmul(
                        pv[0:128, TB:2 * TB], lhsT=vB8[:, pr, :, :],
                        rhs=e2[:, :, 1, :],
                        start=(pr == 0), stop=False,
                        perf_mode=mybir.MatmulPerfMode.DoubleRowSwInterleave,
                        skip_group_check=True,
                    )

                # diagonal kv tiles: bf16 probs, causal column trim + triangle mask
                for jl in range(4):
                    jt = ib * 4 + jl
                    c0 = jl * 128
                    jsl = slice(jt * 128, (jt + 1) * 128)
                    s = ps_b.tile([128, 2 * TB], F32, tag="s")
                    nc.tensor.matmul(
                        s[:, c0:TB], lhsT=kT[0:64, jsl], rhs=qt[0:64, c0:TB],
                        start=True, stop=True, tile_position=(0, 0),
                    )
                    nc.tensor.matmul(
                        s[:, TB + c0:2 * TB], lhsT=kT[64:128, jsl], rhs=qt[64:128, c0:TB],
                        start=True, stop=True, tile_position=(64, 0),
                    )
                    e = exp_pool.tile([128, 2 * TB], BF16, tag="e")
                    nc.scalar.activation(
                        e[:, c0:2 * TB], s[:, c0:2 * TB],
                        mybir.ActivationFunctionType.Exp,
                        bias=ebias[:], scale=SCALE,
                    )
                    with nc.allow_low_precision(reason="exact 0/1 mask on bf16 probs"):
                        nc.vector.tensor_mul(
                            e[:, c0:c0 + 128], e[:, c0:c0 + 128], tri_sb[:])
                        nc.vector.tensor_mul(
                            e[:, TB + c0:TB + c0 + 128],
                            e[:, TB + c0:TB + c0 + 128], tri_sb[:])
                    first = (ib == 0 and jl == 0)
                    nc.tensor.matmul(
                        pv[0:65, c0:TB], lhsT=v_nat[:, jt, 0:65], rhs=e[:, c0:TB],
                        start=first, stop=(jl == 3), skip_group_check=True,
                    )
                    nc.tensor.matmul(
                        pv[0:65, TB + c0:2 * TB], lhsT=v_nat[:, jt, 65:130],
                        rhs=e[:, TB + c0:2 * TB],
                        start=first, stop=(jl == 3), skip_group_check=True,
                    )

                return pv

            def attn_norm(pv, b, ib, half):
                # normalize both heads and ship to the AG input for this half
                blk = half * 8 + b * 2 + ib // 2
                srow = sr_pool.tile([1, 2 * TB], F32, tag="sr")
                nc.vector.tensor_copy(srow[:], pv[64:65, :])
                r32 = sr_pool.tile([1, 2 * TB], F32, tag="sr")
                nc.vector.reciprocal_approx_fast(out=r32[:], in_=srow[:])
                nc.sync.dma_start(out=rnorm_d.ap()[blk, :], in_=r32[:])
                rb = evac_pool.tile([64, 2 * TB], F32, tag="rb")
                base = rnorm_d.ap()[blk, :]
                rb_src = bass.AP(
                    tensor=base.tensor,
                    offset=base.offset,
                    ap=[[0, 64]] + [list(p) for p in base.ap],
                )
                nc.sync.dma_start(out=rb[:], in_=rb_src)
                outn = evac_pool.tile([64, 2 * TB], BF16, tag="on")
                with nc.allow_low_precision(reason="normalized attn out as bf16"):
                    nc.vector.tensor_mul(outn[:], pv[0:64, :], rb[:])
                chunk = b * 2 + ib // 2
                nc.gpsimd.dma_start(out=a2i[half].ap()[chunk, 0:64, :], in_=outn[:, 0:TB])
                nc.gpsimd.dma_start(out=a2i[half].ap()[chunk, 64:128, :], in_=outn[:, TB:2 * TB])

            for b in range(B):
                pv0 = attn_compute(b, 0, 0)
                pv2 = attn_compute(b, 2, 0)
                attn_norm(pv0, b, 0, 0)
                attn_norm(pv2, b, 2, 0)
            nc.gpsimd.collective_compute(
                "AllToAll", mybir.AluOpType.bypass,
                ins=[a2i[0].ap()], outs=[a2o[0].ap()],
                replica_groups=[list(range(N_CORES))],
            )
            for b in range(B):
                pv1 = attn_compute(b, 1, 1)
                pv3 = attn_compute(b, 3, 1)
                attn_norm(pv1, b, 1, 1)
                attn_norm(pv3, b, 3, 1)

            # ---- output projection for my 1024-token slice ----
            for half in range(2):
                if half == 1:
                    nc.gpsimd.collective_compute(
                        "AllToAll", mybir.AluOpType.bypass,
                        ins=[a2i[1].ap()], outs=[a2o[1].ap()],
                        replica_groups=[list(range(N_CORES))],
                    )
                at = xt_pool.tile([128, NKT, TB], BF16, tag="xt")
                for kt in range(NKT):
                    nc.gpsimd.dma_start(
                        out=at[:, kt, :],
                        in_=a2o[half].ap()[kt, :, :],
                    )
                for dt in range(8):
                    ps = ps_a.tile([128, TB], F32, tag="a")
                    for kt in range(NKT):
                        nc.tensor.matmul(
                            ps[:],
                            lhsT=wproj_sb[:, kt, dt * 128:(dt + 1) * 128],
                            rhs=at[:, kt, :],
                            start=(kt == 0),
                            stop=(kt == NKT - 1),
                        )
                    ot = po_pool.tile([128, TB], F32, tag="po")
                    nc.scalar.activation(
                        ot[:], ps[:], mybir.ActivationFunctionType.Identity,
                        bias=bmat_sb[:, dt:dt + 1], scale=1.0,
                    )
                    nc.sync.dma_start(
                        out=outT.ap()[dt * 128:(dt + 1) * 128, half * TB:(half + 1) * TB],
                        in_=ot[:],
                    )

    nc.compile()
    return nc


_NC = None
_last_in_maps = None


def _get_nc():
    global _NC
    if _NC is None:
        _NC = build()
    return _NC


def kernel(x, w_qkv, w_proj, b_proj):
    nc = _get_nc()

    x = np.asarray(x, dtype=np.float32)
    w_qkv = np.asarray(w_qkv, dtype=np.float32)
    w_proj = np.asarray(w_proj, dtype=np.float32)
    b_proj = np.asarray(b_proj, dtype=np.float32)

    xT = np.ascontiguousarray(x.reshape(BT, C).T).astype(ml_dtypes.bfloat16)
    wprojT = np.ascontiguousarray(w_proj.T).astype(ml_dtypes.bfloat16)
    bmat = np.ascontiguousarray(b_proj.reshape(8, 128).T)
    p = np.arange(128)[:, None]
    f = np.arange(128)[None, :]
    tri = (p <= f).astype(ml_dtypes.bfloat16)
    ident = np.eye(128, dtype=np.float32).astype(ml_dtypes.bfloat16)
    ident8 = np.eye(128, dtype=np.float32)[:, ::-1].astype(ml_dtypes.float8_e4m3fn)

    in_maps = []
    for c in range(N_CORES):
        rows = slice(c * 128, (c + 1) * 128)
        w_local = np.concatenate(
            [w_qkv[0:C][rows], w_qkv[C:2 * C][rows], w_qkv[2 * C:3 * C][rows]], axis=0
        )  # [384, C]
        in_maps.append({
            "xT": xT,
            "wqkvT": np.ascontiguousarray(w_local.T).astype(ml_dtypes.bfloat16),
            "wprojT": wprojT,
            "bmat": bmat,
            "tri": tri,
            "ident": ident,
            "ident8": ident8,
        })

    global _last_in_maps
    _last_in_maps = in_maps
    res = run_bass_kernel_spmd(nc, in_maps, core_ids=list(range(N_CORES)))
    outT_full = np.concatenate([res.results[c]["outT"] for c in range(N_CORES)], axis=1)
    return np.ascontiguousarray(outT_full.T).reshape(B, T, C)
